# revision 1
# baseline (speedup 1.0000x reference)
"""Multi-head attention Bass/Tile kernel for Trainium2, 8-core SPMD.

Sharding: core c -> (batch b=c//2, query-half qh=c%2). Each core computes a
disjoint [1024, 512] slab of the output; no collectives needed.

Host prep per core (numpy):
  - gather unmasked keys of value[b] (mask==1), pad to S_K=1280 with zeros
  - transpose activations to [D, S] layout (matmul wants contraction on
    partitions), append a ones-row for bias handling
  - zero the columns of masked query rows: the reference's additive -1e9
    query mask absorbs all score bits in fp32, making masked rows a uniform
    average over unmasked keys -- a zero query produces exactly that
  - fold 1/sqrt(DK) into Wq; stack biases under weights; interleave Wv with a
    "valid" indicator column per head (so the softmax denominator comes out
    of the ctx matmul as a 65th row)

Device (per core): QKV projections (transposed layout) -> per head:
scores^T[key,q] = k^T.T @ q^T -> exp on ACT -> ctx^T[dv,q] accumulated over
key blocks with denominator row -> normalize -> out = ctx^T.T @ Wo.
"""

import sys
import numpy as np

for p in ("/opt/trn_rl_repo",):
    if p not in sys.path:
        sys.path.insert(0, p)

import ml_dtypes

BF16 = ml_dtypes.bfloat16

B, S, D = 4, 2048, 512
H, DK, DV = 8, 64, 64
SQ = 1024          # query rows per core
SK = 1280          # padded gathered-key count (>= max unmasked keys, ~1024)
KB = SK // 128     # key blocks
NCORES = 8

_prog = None        # cached (nc, names)
LAST_EXEC_NS = None
LAST_PROFILE = None


def _build_program():
    from contextlib import ExitStack
    import concourse.bass as bass
    import concourse.mybir as mybir

    f32 = mybir.dt.float32
    bf16 = mybir.dt.bfloat16
    Exp = mybir.ActivationFunctionType.Exp

    nc = bass.Bass()

    xqT_d = nc.declare_dram_parameter("xqT", [128, 4 * SQ], bf16, isOutput=False)
    xqr_d = nc.declare_dram_parameter("xqr", [1, SQ], bf16, isOutput=False)
    xvT_d = nc.declare_dram_parameter("xvT", [128, 4 * SK], bf16, isOutput=False)
    xvr_d = nc.declare_dram_parameter("xvr", [1, SK], bf16, isOutput=False)
    wq_d = nc.declare_dram_parameter("wq", [128, 2048], bf16, isOutput=False)
    wqr_d = nc.declare_dram_parameter("wqr", [1, 512], bf16, isOutput=False)
    wk_d = nc.declare_dram_parameter("wk", [128, 2048], bf16, isOutput=False)
    wkr_d = nc.declare_dram_parameter("wkr", [1, 512], bf16, isOutput=False)
    wv_d = nc.declare_dram_parameter("wv", [128, 2080], bf16, isOutput=False)
    wvr_d = nc.declare_dram_parameter("wvr", [1, 520], bf16, isOutput=False)
    wo_d = nc.declare_dram_parameter("wo", [128, 2048], bf16, isOutput=False)
    sel_d = nc.declare_dram_parameter("sel", [8, 512], f32, isOutput=False)
    out_d = nc.declare_dram_parameter("out", [SQ, 512], f32, isOutput=True)

    es = ExitStack()
    with es:
        _n = [0]
        def sb(shape, dt):
            _n[0] += 1
            return es.enter_context(nc.sbuf_tensor(f"t{_n[0]}", shape, dt))
        xq_t = sb([128, 4 * SQ], bf16); xq_r = sb([1, SQ], bf16)
        xv_t = sb([128, 4 * SK], bf16); xv_r = sb([1, SK], bf16)
        wq_t = sb([128, 2048], bf16); wq_r = sb([1, 512], bf16)
        wk_t = sb([128, 2048], bf16); wk_r = sb([1, 512], bf16)
        wv_t = sb([128, 2080], bf16); wv_r = sb([1, 520], bf16)
        wo_t = sb([128, 2048], bf16)
        qT = [sb([128, SQ], bf16) for _ in range(4)]
        kT = [sb([128, SK], bf16) for _ in range(4)]
        vv = [sb([128, 520], bf16) for _ in range(KB)]
        pT = [sb([128, SQ], bf16) for _ in range(4)]
        ctxT = [sb([128, SQ], bf16) for _ in range(4)]
        dH = [sb([1, SQ], f32) for _ in range(8)]
        rH = [sb([1, SQ], f32) for _ in range(8)]
        sel_t = sb([8, 512], f32)
        bcs2 = sb([128, SQ], f32)
        ctxTn = [sb([128, SQ], bf16) for _ in range(4)]
        osb = [sb([128, 512], f32) for _ in range(2)]

        dma_s = es.enter_context(nc.semaphore("dma_s"))
        pe_s = es.enter_context(nc.semaphore("pe_s"))
        act_s = es.enter_context(nc.semaphore("act_s"))
        dve_s = es.enter_context(nc.semaphore("dve_s"))
        pool_s = es.enter_context(nc.semaphore("pool_s"))

        xq = [xq_t[:, i * SQ:(i + 1) * SQ] for i in range(4)]
        xv = [xv_t[:, i * SK:(i + 1) * SK] for i in range(4)]
        wq = [wq_t[:, i * 512:(i + 1) * 512] for i in range(4)]
        wk = [wk_t[:, i * 512:(i + 1) * 512] for i in range(4)]
        wv = [wv_t[:, i * 520:(i + 1) * 520] for i in range(4)]
        wo = [wo_t[:, i * 512:(i + 1) * 512] for i in range(4)]

        NPROJ = 18           # 4 qT + 4 kT + KB v projection groups
        # pe milestones: phase1 group g done -> g+1
        # attention (p,kb): scores h2 -> 18+p*40+kb*4+h2+1 ; ctx-final h2 ->
        # 18+p*40+kb*4+2+h2+1 ; out qb -> 178+qb+1
        pe_sc = lambda p, kb, h2: NPROJ + p * 4 * KB + kb * 4 + h2 + 1
        pe_cx = lambda p, kb, h2: NPROJ + p * 4 * KB + kb * 4 + 2 + h2 + 1
        PE_ATT = NPROJ + 16 * KB
        # dve: phase1 copies -> 18 ; normalize (copy,recip,mult)x2 per pair ;
        # out copies
        dve_evac = lambda p, h2: NPROJ + p * 6 + h2 * 3 + 2
        DVE_NRM = NPROJ + 24
        # act: exp (p,kb,h2) -> s_idx+1
        s_of = lambda p, kb, h2: p * 2 * KB + kb * 2 + h2

        # ---------------- phase 1: load + projections -------------------
        with (
            nc.psum_tensor("pp0", [128, SK], f32) as pp0,
            nc.psum_tensor("pp1", [128, SK], f32) as pp1,
            nc.Block() as blk,
        ):
            pp = [pp0, pp1]

            @blk.sync
            def _(sync):
                for t, d in ((xq_t, xqT_d), (xq_r, xqr_d), (xv_t, xvT_d),
                             (xv_r, xvr_d), (wq_t, wq_d), (wq_r, wqr_d),
                             (wk_t, wk_d), (wk_r, wkr_d), (wv_t, wv_d),
                             (wv_r, wvr_d), (wo_t, wo_d), (sel_t, sel_d)):
                    sync.dma_start(t[:], d[:]).then_inc(dma_s, 16)

            @blk.tensor
            def _(te):
                te.wait_ge(dma_s, 192)
                g = 0
                for ft in range(4):                       # qT
                    ps = pp[g % 2]
                    if g >= 2:
                        te.wait_ge(dve_s, g - 1)
                    fsl = slice(ft * 128, (ft + 1) * 128)
                    last = None
                    for nh in range(SQ // 512):
                        nsl = slice(nh * 512, (nh + 1) * 512)
                        for ci in range(4):
                            te.matmul(ps[:, nsl], wq[ci][:, fsl],
                                      xq[ci][:, nsl],
                                      start=(ci == 0), stop=False)
                        last = te.matmul(ps[:, nsl], wq_r[:, fsl],
                                         xq_r[:, nsl], start=False, stop=True)
                    last.then_inc(pe_s, 1)
                    g += 1
                for ft in range(4):                       # kT
                    ps = pp[g % 2]
                    te.wait_ge(dve_s, g - 1)
                    fsl = slice(ft * 128, (ft + 1) * 128)
                    last = None
                    for (off, w) in ((0, 512), (512, 512), (1024, 256)):
                        nsl = slice(off, off + w)
                        for ci in range(4):
                            te.matmul(ps[:, nsl], wk[ci][:, fsl],
                                      xv[ci][:, nsl],
                                      start=(ci == 0), stop=False)
                        last = te.matmul(ps[:, nsl], wk_r[:, fsl],
                                         xv_r[:, nsl], start=False, stop=True)
                    last.then_inc(pe_s, 1)
                    g += 1
                for rb in range(KB):                      # v
                    ps = pp[g % 2]
                    te.wait_ge(dve_s, g - 1)
                    rsl = slice(rb * 128, (rb + 1) * 128)
                    last = None
                    for (off, w) in ((0, 512), (512, 8)):
                        nsl = slice(off, off + w)
                        for ci in range(4):
                            te.matmul(ps[:, nsl], xv[ci][:, rsl],
                                      wv[ci][:, nsl],
                                      start=(ci == 0), stop=False)
                        last = te.matmul(ps[:, nsl], xv_r[:, rsl],
                                         wv_r[:, nsl], start=False, stop=True)
                    last.then_inc(pe_s, 1)
                    g += 1

            @blk.vector
            def _(ve):
                g = 0
                for ft in range(4):
                    ve.wait_ge(pe_s, g + 1)
                    ve.tensor_copy(qT[ft][:], pp[g % 2][:, 0:SQ]).then_inc(dve_s, 1)
                    g += 1
                for ft in range(4):
                    ve.wait_ge(pe_s, g + 1)
                    ve.tensor_copy(kT[ft][:], pp[g % 2][:, 0:SK]).then_inc(dve_s, 1)
                    g += 1
                for rb in range(KB):
                    ve.wait_ge(pe_s, g + 1)
                    ve.tensor_copy(vv[rb][:], pp[g % 2][:, 0:520]).then_inc(dve_s, 1)
                    g += 1

        # ---------------- phase 2: attention + output -------------------
        with (
            nc.psum_tensor("sc0", [128, SQ], f32) as sc0,
            nc.psum_tensor("sc1", [128, SQ], f32) as sc1,
            nc.psum_tensor("cx0", [65, SQ], f32) as cx0,
            nc.psum_tensor("cx1", [65, SQ], f32) as cx1,
            nc.Block() as blk2,
        ):
            sc = [sc0, sc1]
            cx = [cx0, cx1]

            @blk2.tensor
            def _(te):
                te.wait_ge(dve_s, NPROJ)
                for p in range(4):
                    for kb in range(KB):
                        ksl = slice(kb * 128, (kb + 1) * 128)
                        for h2 in range(2):
                            s = s_of(p, kb, h2)
                            psl = slice(h2 * 64, (h2 + 1) * 64)
                            if s >= 2:
                                te.wait_ge(act_s, s - 1)
                            for nh in range(2):
                                nsl = slice(nh * 512, (nh + 1) * 512)
                                ins = te.matmul(sc[s % 2][:, nsl],
                                                kT[p][psl, ksl],
                                                qT[p][psl, nsl],
                                                start=True, stop=True)
                            ins.then_inc(pe_s, 1)
                        for h2 in range(2):
                            s = s_of(p, kb, h2)
                            if kb == 0 and p > 0:
                                te.wait_ge(dve_s, dve_evac(p - 1, h2))
                            te.wait_ge(act_s, s + 1)
                            vh = vv[kb][:, (2 * p + h2) * 65:(2 * p + h2 + 1) * 65]
                            for nh in range(2):
                                nsl = slice(nh * 512, (nh + 1) * 512)
                                ins = te.matmul(cx[h2][:, nsl], vh,
                                                pT[s % 4][:, nsl],
                                                start=(kb == 0),
                                                stop=(kb == KB - 1),
                                                skip_group_check=True)
                            ins.then_inc(pe_s, 1)
            @blk2.scalar
            def _(ac):
                for p in range(4):
                    for kb in range(KB):
                        for h2 in range(2):
                            s = s_of(p, kb, h2)
                            if s >= 4:
                                sp, r = divmod(s - 4, 2 * KB)
                                ac.wait_ge(pe_s, pe_cx(sp, r // 2, r % 2))
                            ac.wait_ge(pe_s, pe_sc(p, kb, h2))
                            ac.activation(pT[s % 4][:], sc[s % 2][:], Exp
                                          ).then_inc(act_s, 1)

            @blk2.vector
            def _(ve):
                for p in range(4):                        # evacuate ctx + denom
                    for h2 in range(2):
                        hh = 2 * p + h2
                        ve.wait_ge(pe_s, pe_cx(p, KB - 1, h2))
                        ve.tensor_copy(ctxT[p][h2 * 64:(h2 + 1) * 64, :],
                                       cx[h2][0:64, :]).then_inc(dve_s, 1)
                        ve.tensor_copy(dH[hh][:], cx[h2][64:65, :]).then_inc(dve_s, 1)
                        ve.reciprocal(rH[hh][:], dH[hh][:]).then_inc(dve_s, 1)

        # ---------------- phase 2b: normalize + output projection -------
        # recip of all 8 denom rows at once; per head: one-hot selector
        # matmul broadcasts the recip row across 64 partitions via PSUM,
        # DVE copies it to SBUF and multiplies into normalized ctxT.
        with (
            nc.psum_tensor("ops0", [128, 512], f32) as ops0,
            nc.psum_tensor("ops1", [128, 512], f32) as ops1,
            nc.psum_tensor("bcp0", [64, SQ], f32) as bcp0,
            nc.psum_tensor("bcp1", [64, SQ], f32) as bcp1,
            nc.Block() as blk3,
        ):
            opsl = [ops0, ops1]
            bcp = [bcp0, bcp1]
            PE_BC = PE_ATT          # 178; bcp MM hh -> +hh+1 ; out qb -> 186+qb+1
            DVE_R = DVE_NRM         # all evac+recip done at 42

            @blk3.tensor
            def _(te):
                te.wait_ge(dve_s, DVE_R)
                for hh in range(8):
                    if hh >= 2:
                        te.wait_ge(dve_s, DVE_R + (hh - 2) * 2 + 1)
                    last = None
                    for nh in range(2):
                        nsl = slice(nh * 512, (nh + 1) * 512)
                        last = te.matmul(bcp[hh % 2][:, nsl],
                                         sel_t[0:1, 0:64],
                                         rH[hh][:, nsl], start=True, stop=True)
                    last.then_inc(pe_s, 1)
                for qb in range(8):
                    qsl = slice(qb * 128, (qb + 1) * 128)
                    if qb == 0:
                        te.wait_ge(dve_s, DVE_R + 16)
                    if qb >= 2:
                        te.wait_ge(dve_s, DVE_R + 16 + qb - 1)
                    last = None
                    for p in range(4):
                        last = te.matmul(opsl[qb % 2][:], ctxTn[p][:, qsl],
                                         wo[p][:], start=(p == 0), stop=(p == 3))
                    last.then_inc(pe_s, 1)

            @blk3.vector
            def _(ve):
                for hh in range(8):
                    p, h2 = hh // 2, hh % 2
                    ve.wait_ge(pe_s, PE_BC + hh + 1)
                    ve.tensor_copy(bcs2[h2 * 64:(h2 + 1) * 64, :],
                                   bcp[hh % 2][:]).then_inc(dve_s, 1)
                    ve.tensor_mul(ctxTn[p][h2 * 64:(h2 + 1) * 64, :],
                                  ctxT[p][h2 * 64:(h2 + 1) * 64, :],
                                  bcs2[h2 * 64:(h2 + 1) * 64, :]).then_inc(dve_s, 1)
                for qb in range(8):
                    ve.wait_ge(pe_s, PE_BC + 8 + qb + 1)
                    if qb >= 2:
                        ve.wait_ge(dma_s, 192 + (qb - 1) * 16)
                    ve.tensor_copy(osb[qb % 2][:], opsl[qb % 2][:]).then_inc(dve_s, 1)

            @blk3.sync
            def _(sync):
                for qb in range(8):
                    sync.wait_ge(dve_s, DVE_R + 16 + qb + 1)
                    sync.dma_start(out_d[qb * 128:(qb + 1) * 128, :],
                                   osb[qb % 2][:]).then_inc(dma_s, 16)
                sync.wait_ge(dma_s, 192 + 128)

    return nc


def _get_program():
    global _prog
    if _prog is None:
        _prog = _build_program()
    return _prog


def kernel(query, value, attention_mask, Wq, bq, Wk, bk, Wv, bv, Wo, bo):
    global LAST_EXEC_NS, LAST_PROFILE
    from concourse.bass_utils import run_bass_kernel_spmd

    query = np.asarray(query, np.float32)
    value = np.asarray(value, np.float32)
    attention_mask = np.asarray(attention_mask)
    Wq = np.asarray(Wq, np.float32); bq = np.asarray(bq, np.float32)
    Wk = np.asarray(Wk, np.float32); bk = np.asarray(bk, np.float32)
    Wv = np.asarray(Wv, np.float32); bv = np.asarray(bv, np.float32)
    Wo = np.asarray(Wo, np.float32); bo = np.asarray(bo, np.float32)

    def pack4(a):  # [512, N] -> [128, 4N] chunk-major
        n = a.shape[1]
        return a.reshape(4, 128, n).transpose(1, 0, 2).reshape(128, 4 * n)

    sc = np.float32(1.0 / np.sqrt(DK))
    wq_aug = pack4(Wq * sc).astype(BF16)
    wqr = (bq[None, :] * sc).astype(BF16)
    wk_aug = pack4(Wk).astype(BF16)
    wkr = bk[None, :].astype(BF16)
    wv_full = np.zeros((513, 520), np.float32)
    for h in range(H):
        wv_full[0:512, h * 65:h * 65 + 64] = Wv[:, h * 64:(h + 1) * 64]
        wv_full[512, h * 65:h * 65 + 64] = bv[h * 64:(h + 1) * 64]
        wv_full[512, h * 65 + 64] = 1.0
    wv_aug = pack4(wv_full[0:512]).astype(BF16)
    wvr = wv_full[512:513].astype(BF16)
    wo_b = pack4(Wo).astype(BF16)
    sel_np = np.zeros((8, 512), np.float32)
    for h in range(H):
        sel_np[h, h * 64:(h + 1) * 64] = 1.0

    in_maps = []
    for c in range(NCORES):
        b, qh = c // 2, c % 2
        m = attention_mask[b]
        idx = np.nonzero(m != 0)[0]
        if len(idx) > SK:
            raise RuntimeError(f"unmasked keys {len(idx)} > SK={SK}")
        xv = np.zeros((512, SK), np.float32)
        xv[:, :len(idx)] = value[b][idx].T
        xvr = np.zeros((1, SK), np.float32)
        xvr[0, :len(idx)] = 1.0
        xq = query[b, qh * SQ:(qh + 1) * SQ].T.copy()
        xqr = np.ones((1, SQ), np.float32)
        mq = np.asarray(m[qh * SQ:(qh + 1) * SQ]) == 0
        xq[:, mq] = 0.0
        xqr[0, mq] = 0.0
        in_maps.append({
            "xqT": pack4(xq).astype(BF16), "xqr": xqr.astype(BF16),
            "xvT": pack4(xv).astype(BF16), "xvr": xvr.astype(BF16),
            "wq": wq_aug, "wqr": wqr, "wk": wk_aug, "wkr": wkr,
            "wv": wv_aug, "wvr": wvr, "wo": wo_b, "sel": sel_np,
        })

    nc = _get_program()
    try:
        res = run_bass_kernel_spmd(nc, in_maps, list(range(NCORES)), trace=True)
    except (ModuleNotFoundError, ImportError):
        res = run_bass_kernel_spmd(nc, in_maps, list(range(NCORES)))
    LAST_EXEC_NS = res.exec_time_ns
    LAST_PROFILE = res.profile_json
    out = np.zeros((B, S, D), np.float32)
    for c in range(NCORES):
        b, qh = c // 2, c % 2
        out[b, qh * SQ:(qh + 1) * SQ] = res.results[c]["out"]
    return out + bo[None, None, :]

